# revision 15
# baseline (speedup 1.0000x reference)
"""Trainium2 Bass kernel for nn_DecoderFactoredLSTM.

Factored-LSTM decoder:
  emb = B_w[captions]                                   [B,T,E] -> tokens [T*B, E]
  u   = emb @ (V^T S^T U^T) + bias                      [T*B, 4H]   (gate pre-activations)
  recurrence over T=40 steps (LSTM, no tanh on c for h)
  out = hiddens @ C_w^T + C_b                           [T*B, V]

Sharding: recurrence + pre-projections replicated on all 8 cores (the
sequential recurrence cannot be sharded without per-step collectives,
whose ~20us latency floor x40 steps dwarfs the compute); the vocab
projection (the dominant FLOPs) is sharded 8-way over vocab columns.
All matmuls run as float32r (reduced-precision fp32 PE mode, ~1.4e-4
rel err, 4x faster than exact fp32).

Column layout for gates everywhere (u, W, gate psum): 8 h-blocks of
512 cols, each block = [i|f|o|ctilde] x 128 h-lanes:
  col(g, h) = (h // 128) * 512 + g * 128 + (h % 128)
"""

import sys

if "/opt/trn_rl_repo" not in sys.path:
    sys.path.insert(0, "/opt/trn_rl_repo")

import numpy as np

import concourse.bass as bass
import concourse.mybir as mybir
import concourse.tile as tile
from concourse import bacc
from concourse.bass import ts, ds
from concourse.bass_utils import run_bass_kernel_spmd
from concourse.masks import make_identity

B, T, E, H, F, V = 64, 40, 512, 1024, 512, 32000
NCORES = 8
VS = V // NCORES  # vocab slice per core: 4000
TOK = T * B  # 2560 tokens
MT = TOK // 128  # 20 token tiles
F32 = mybir.dt.float32
F32R = mybir.dt.float32r

PRO_STEPS = 4  # python-unrolled recurrence prologue steps (incl. t=0)


def _r(ap):
    return ap.bitcast(F32R)


def _build():
    nc = bacc.Bacc(None, target_bir_lowering=False, debug=False)

    with tile.TileContext(nc) as tc:
        cap_d = nc.declare_dram_parameter("cap", [TOK, 1], mybir.dt.int32, isOutput=False)
        Bw_d = nc.declare_dram_parameter("Bw", [V, E], F32, isOutput=False)
        Vg_d = nc.declare_dram_parameter("Vg", [4, F, E], F32R, isOutput=False)
        SgT_d = nc.declare_dram_parameter("SgT", [4, F, F], F32R, isOutput=False)
        UgT_d = nc.declare_dram_parameter("UgT", [4, F, H], F32R, isOutput=False)
        W_d = nc.declare_dram_parameter("Wmov", [H, 4 * H], F32R, isOutput=False)
        ub_d = nc.declare_dram_parameter("ubias", [128, 4 * H], F32, isOutput=False)
        CT_d = nc.declare_dram_parameter("CT", [H, VS], F32R, isOutput=False)
        Cb_d = nc.declare_dram_parameter("Cb", [128, VS], F32, isOutput=False)

        out_d = nc.declare_dram_parameter("out", [TOK, VS], F32, isOutput=True)

        embT_d = nc.dram_tensor("embT", [4, 128, TOK], F32R)  # [e_outer, e_inner, tok]
        u_d = nc.dram_tensor("u", [TOK, 4 * H], F32)
        hT_d = nc.dram_tensor("hT", [8, 128, TOK], F32R)  # [k_outer, k_inner, tok]

        with (
            tc.tile_pool(name="const", bufs=1) as const,
            tc.tile_pool(name="pers", bufs=1) as pers,
        ):
            id128 = const.tile([128, 128], F32, tag="id128")
            make_identity(nc, id128)
            id64 = const.tile([64, 64], F32, tag="id64")
            make_identity(nc, id64)
            # W half A (k-tiles 0..3) prefetched at kernel start, overlapping
            # the gather/M/u phases; half B loads when phase-2 SBUF frees.
            wpoolA = tc.tile_pool(name="wpoolA", bufs=1)
            wpA = wpoolA.__enter__()
            wsbA = wpA.tile([128, 4, 4 * H], F32R, tag="wsbA")
            nc.sync.dma_start(
                wsbA[:], W_d[0 : H // 2, :].rearrange("(ko ki) n -> ki ko n", ki=128)
            )

            # ================= phase 1: gather + transpose emb =================
            with (
                tc.tile_pool(name="ph1", bufs=1) as ph1,
                tc.tile_pool(name="ph1ps", bufs=2, space="PSUM") as ph1ps,
            ):
                idx_all = ph1.tile([128, MT], mybir.dt.int32, tag="idx")
                nc.sync.dma_start(
                    idx_all[:], cap_d[:].rearrange("(m p) o -> p (m o)", p=128)
                )
                for m in range(MT):
                    g_t = ph1.tile([128, E], F32, tag=f"g{m % 3}")
                    nc.gpsimd.indirect_dma_start(
                        out=g_t[:],
                        out_offset=None,
                        in_=Bw_d[:],
                        in_offset=bass.IndirectOffsetOnAxis(
                            ap=idx_all[:, m : m + 1], axis=0
                        ),
                    )
                    stg = ph1.tile([128, 4, 128], F32R, tag=f"stg{m % 3}", name=f"stg{m % 3}")
                    for e in range(4):
                        tp = ph1ps.tile([128, 128], F32, tag=f"tp{e % 2}")
                        nc.tensor.transpose(tp[:], g_t[:, ts(e, 128)], id128[:])
                        nc.vector.tensor_copy(stg[:, e, :], tp[:])
                    nc.sync.dma_start(
                        embT_d[:].rearrange("e ki t -> ki e t")[:, :, ts(m, 128)],
                        stg[:],
                    )

            # ================= phase 2: M = V^T S^T U^T, then u = embT^T @ M ==
            with tc.tile_pool(name="ph2", bufs=1) as ph2:
                mcat = ph2.tile([128, 4, 4 * H], F32R, tag="mcat")
                mpre_sb = tc.tile_pool(name="ph2msb", bufs=1)
                ph2m = mpre_sb.__enter__()
                mpre_ps = tc.tile_pool(name="ph2ps", bufs=2, space="PSUM")
                ph2ps = mpre_ps.__enter__()
                for g in range(4):
                    vg = ph2m.tile([128, 4, E], F32R, tag="vg")
                    nc.sync.dma_start(
                        vg[:], Vg_d[g].rearrange("(ko ki) e -> ki ko e", ki=128)
                    )
                    sgT = ph2m.tile([128, 4, F], F32R, tag="sgT")
                    nc.sync.dma_start(
                        sgT[:], SgT_d[g].rearrange("(ko ki) f -> ki ko f", ki=128)
                    )
                    ugT = ph2m.tile([128, 4, H], F32R, tag="ugT")
                    nc.sync.dma_start(
                        ugT[:], UgT_d[g].rearrange("(ko ki) h -> ki ko h", ki=128)
                    )
                    # PT[f', e] = sum_f S[f',f] V[f,e]; lhsT=S^T [f,f'], rhs=V [f,e]
                    pt = ph2m.tile([128, 4, E], F32R, tag="pt")
                    for fp in range(4):
                        ps = ph2ps.tile([128, E], F32, tag="mp")
                        for k in range(4):
                            nc.tensor.matmul(
                                ps[:],
                                lhsT=sgT[:, k, ts(fp, 128)],
                                rhs=vg[:, k, :],
                                start=(k == 0),
                                stop=(k == 3),
                            )
                        nc.vector.tensor_copy(pt[:, fp, :], ps[:])
                    # M[e, h] = sum_f' PT[f',e] U^T[f',h]
                    for e_t in range(4):
                        for nh in range(2):
                            ps2 = ph2ps.tile([128, 512], F32, tag="mp2")
                            for k in range(4):
                                nc.tensor.matmul(
                                    ps2[:],
                                    lhsT=pt[:, k, ts(e_t, 128)],
                                    rhs=ugT[:, k, ts(nh, 512)],
                                    start=(k == 0),
                                    stop=(k == 3),
                                )
                            for b4 in range(4):
                                blk = nh * 4 + b4
                                nc.vector.tensor_copy(
                                    mcat[:, e_t, blk * 512 + g * 128 : blk * 512 + g * 128 + 128],
                                    ps2[:, ts(b4, 128)],
                                )

                mpre_ps.__exit__(None, None, None)
                mpre_sb.__exit__(None, None, None)
                # u-phase: u[tok, col] = sum_e embT[e, tok] M[e, col] + ubias
                u_sb = tc.tile_pool(name="ph2usb", bufs=1)
                ph2u = u_sb.__enter__()
                u_ps = tc.tile_pool(name="ph2psu", bufs=1, space="PSUM")
                ph2psu = u_ps.__enter__()
                ubias = ph2u.tile([128, 4 * H], F32, tag="ubias")
                nc.sync.dma_start(ubias[:], ub_d[:])
                for m in range(MT):
                    lts = []
                    for k in range(4):
                        lt = ph2u.tile(
                            [128, 128], F32R, tag=f"lt{k}_{m % 2}", name=f"lt{k}_{m % 2}"
                        )
                        nc.scalar.dma_start(lt[:], embT_d[k, :, ts(m, 128)])
                        lts.append(lt)
                    pss = []
                    for n in range(8):
                        pss.append(ph2psu.tile([128, 512], F32, tag=f"up{n}", name=f"up{n}"))
                    for k in range(4):
                        for n in range(8):
                            nc.tensor.matmul(
                                pss[n][:],
                                lhsT=lts[k][:],
                                rhs=mcat[:, k, ts(n, 512)],
                                start=(k == 0),
                                stop=(k == 3),
                            )
                    uev = ph2u.tile([128, 4 * H], F32, tag=f"uev{m % 2}", name=f"uev{m % 2}")
                    for n in range(8):
                        nc.vector.tensor_add(
                            uev[:, ts(n, 512)], pss[n][:], ubias[:, ts(n, 512)]
                        )
                    nc.sync.dma_start(u_d[ts(m, 128), :], uev[:])
                u_ps.__exit__(None, None, None)
                u_sb.__exit__(None, None, None)

            # ================= phase 3: recurrence =================
            with (
                tc.tile_pool(name="ph3", bufs=1) as ph3,
                tc.tile_pool(name="ph3ps", bufs=1, space="PSUM") as ph3ps,
                tc.tile_pool(name="ph3pst", bufs=2, space="PSUM") as ph3pst,
            ):
                wsbB = ph3.tile([128, 4, 4 * H], F32R, tag="wsbB")

                def load_wsbB():
                    for wc in range(2):
                        nc.scalar.dma_start(
                            wsbB[:, ts(wc, 2), :],
                            W_d[
                                H // 2 + wc * 256 : H // 2 + (wc + 1) * 256, :
                            ].rearrange("(ko ki) n -> ki ko n", ki=128),
                        )
                c_sb = pers.tile([64, H], F32, tag="c")
                hidT = [
                    pers.tile([128, 8, 64], F32R, tag="hidTa", name="hidTa"),
                    pers.tile([128, 8, 64], F32R, tag="hidTb", name="hidTb"),
                ]

                def step(t_first, u_slice_rows, hT_cols, parity):
                    """One LSTM step. u_slice_rows/hT_cols: functions giving
                    the dynamic slices; parity: read hidT[1-p], write hidT[p]."""
                    u_t = ph3.tile([64, 4 * H], F32, tag=f"ut{parity}")
                    nc.sync.dma_start(u_t[:], u_d[u_slice_rows, :])
                    rd = hidT[1 - parity]
                    wr = hidT[parity]
                    for nb in range(8):
                        if t_first:
                            src = u_t[:, ts(nb, 512)]
                        else:
                            ps = ph3ps.tile([64, 512], F32, tag=f"rp{nb % 4}")
                            for k in range(8):
                                wtile = wsbA if k < 4 else wsbB
                                nc.tensor.matmul(
                                    ps[:],
                                    lhsT=rd[:, k, :],
                                    rhs=wtile[:, k % 4, ts(nb, 512)],
                                    start=(k == 0),
                                    stop=(k == 7),
                                )
                            gs = ph3.tile([64, 512], F32, tag=f"gs{nb % 2}")
                            nc.vector.tensor_add(gs[:], ps[:], u_t[:, ts(nb, 512)])
                            src = gs[:]
                        sio = ph3.tile([64, 384], F32, tag=f"sio{nb % 2}")
                        nc.scalar.activation(
                            sio[:], src[:, 0:384], mybir.ActivationFunctionType.Sigmoid
                        )
                        tt = ph3.tile([64, 128], F32, tag=f"tt{nb % 2}")
                        nc.scalar.activation(
                            tt[:], src[:, 384:512], mybir.ActivationFunctionType.Tanh
                        )
                        it = ph3.tile([64, 128], F32, tag=f"it{nb % 2}")
                        nc.vector.tensor_mul(it[:], sio[:, 0:128], tt[:])
                        if t_first:
                            nc.vector.tensor_copy(c_sb[:, ts(nb, 128)], it[:])
                        else:
                            fc = ph3.tile([64, 128], F32, tag=f"fc{nb % 2}")
                            nc.vector.tensor_mul(
                                fc[:], sio[:, 128:256], c_sb[:, ts(nb, 128)]
                            )
                            nc.vector.tensor_add(c_sb[:, ts(nb, 128)], fc[:], it[:])
                        hb = ph3.tile([64, 128], F32, tag=f"hb{nb % 2}", name=f"hb{nb % 2}")
                        nc.vector.tensor_mul(hb[:], sio[:, 256:384], c_sb[:, ts(nb, 128)])
                        tp = ph3pst.tile([128, 64], F32, tag="tp64")
                        nc.tensor.transpose(tp[:], hb[:], id64[:])
                        nc.vector.tensor_copy(wr[:, nb, :], tp[:])
                    # one DMA out per step: hT_d[ko, ki, tok_cols] <- wr [ki, ko, b]
                    nc.sync.dma_start(
                        hT_d[:].rearrange("ko ki t -> ki ko t")[:, :, hT_cols],
                        wr[:],
                    )

                for t in range(PRO_STEPS):
                    step(t == 0, slice(t * 64, (t + 1) * 64), slice(t * 64, (t + 1) * 64), t % 2)
                    if t == 0:
                        load_wsbB()
                UNROLL = 12
                n_iters = (T - PRO_STEPS) // UNROLL
                with tc.For_i(
                    0, n_iters, 1, hint_engines=(mybir.EngineType.PE,)
                ) as iv:
                    for j in range(UNROLL):
                        tj = PRO_STEPS + j
                        step(
                            False,
                            ds(iv * (UNROLL * 64) + tj * 64, 64),
                            ds(iv * (UNROLL * 64) + tj * 64, 64),
                            tj % 2,
                        )

            wpoolA.__exit__(None, None, None)

            # ================= phase 4: vocab projection =================
            with (
                tc.tile_pool(name="ph4", bufs=1) as ph4,
                tc.tile_pool(name="ph4ps", bufs=1, space="PSUM") as ph4ps,
            ):
                ctA = ph4.tile([128, 8, VS // 2], F32R, tag="ctA")
                ctB = ph4.tile([128, 8, VS // 2], F32R, tag="ctB")
                Q = VS // 4  # 1000 cols per chunk
                for q in range(4):
                    cth_ = ctA if q < 2 else ctB
                    nc.scalar.dma_start(
                        cth_[:, :, ts(q % 2, Q)],
                        CT_d[:, q * Q : (q + 1) * Q].rearrange(
                            "(ko ki) n -> ki ko n", ki=128
                        ),
                    )
                cb = ph4.tile([128, VS], F32, tag="cb")
                nc.sync.dma_start(cb[:], Cb_d[:])
                NP = VS // 8  # 500
                for m in range(MT):
                    lts = []
                    for k in range(8):
                        lt = ph4.tile(
                            [128, 128], F32R, tag=f"plt{k}_{m % 2}", name=f"plt{k}_{m % 2}"
                        )
                        nc.scalar.dma_start(lt[:], hT_d[k, :, ts(m, 128)])
                        lts.append(lt)
                    pss = []
                    for n in range(8):
                        pss.append(ph4ps.tile([128, NP], F32, tag=f"pp{n}", name=f"pp{n}"))
                    for k in range(8):
                        for n in range(8):
                            cth = ctA if n < 4 else ctB
                            nc.tensor.matmul(
                                pss[n][:],
                                lhsT=lts[k][:],
                                rhs=cth[:, k, ts(n % 4, NP)],
                                start=(k == 0),
                                stop=(k == 7),
                            )
                    pev = ph4.tile([128, VS], F32, tag=f"pev{m % 2}")
                    for n in range(8):
                        nc.vector.tensor_add(
                            pev[:, ts(n, NP)], pss[n][:], cb[:, ts(n, NP)]
                        )
                    nc.sync.dma_start(out_d[ts(m, 128), :], pev[:])

    nc.compile()
    return nc


def kernel(**inputs):
    captions = np.asarray(inputs["captions"])
    B_w = np.asarray(inputs["B_w"], dtype=np.float32)
    V_w = np.asarray(inputs["V_w"], dtype=np.float32)
    V_b = np.asarray(inputs["V_b"], dtype=np.float32)
    S_w = np.asarray(inputs["S_w"], dtype=np.float32)
    S_b = np.asarray(inputs["S_b"], dtype=np.float32)
    U_w = np.asarray(inputs["U_w"], dtype=np.float32)
    U_b = np.asarray(inputs["U_b"], dtype=np.float32)
    W_w = np.asarray(inputs["W_w"], dtype=np.float32)
    W_b = np.asarray(inputs["W_b"], dtype=np.float32)
    C_w = np.asarray(inputs["C_w"], dtype=np.float32)
    C_b = np.asarray(inputs["C_b"], dtype=np.float32)

    # --- host-side layout prep (weights only) ---
    cap = np.ascontiguousarray(captions.T.reshape(TOK, 1)).astype(np.int32)
    SgT = np.ascontiguousarray(S_w.transpose(0, 2, 1))
    UgT = np.ascontiguousarray(U_w.transpose(0, 2, 1))
    # Wmov[k, col(g,h)]: [4,H,K] -> [K, 8, 4, 128] -> [K, 4H]
    Wmov = np.ascontiguousarray(
        W_w.transpose(2, 0, 1).reshape(H, 4, 8, 128).transpose(0, 2, 1, 3).reshape(H, 4 * H)
    )
    # gate bias chain, folded: ((V_b @ S^T + S_b) @ U^T + U_b) + W_b
    bs = np.einsum("gf,gof->go", V_b, S_w) + S_b  # [4, F]
    bu = np.einsum("gf,ghf->gh", bs, U_w) + U_b  # [4, H]
    gate_bias = bu + W_b  # [4, H]
    ub_cols = gate_bias.reshape(4, 8, 128).transpose(1, 0, 2).reshape(4 * H)
    ub_rep = np.ascontiguousarray(np.broadcast_to(ub_cols, (128, 4 * H)))
    CT = np.ascontiguousarray(C_w.T)  # [H, V]

    nc = _build()

    in_maps = []
    for c in range(NCORES):
        in_maps.append(
            {
                "cap": cap,
                "Bw": B_w,
                "Vg": V_w,
                "SgT": SgT,
                "UgT": UgT,
                "Wmov": Wmov,
                "ubias": ub_rep,
                "CT": np.ascontiguousarray(CT[:, c * VS : (c + 1) * VS]),
                "Cb": np.ascontiguousarray(
                    np.broadcast_to(C_b[c * VS : (c + 1) * VS], (128, VS))
                ),
            }
        )

    global _last_in_maps
    _last_in_maps = in_maps

    res = run_bass_kernel_spmd(nc, in_maps, list(range(NCORES)))
    out = np.concatenate([res.results[c]["out"] for c in range(NCORES)], axis=1)
    return out.astype(np.float32)


_last_in_maps = None
